# revision 4
# baseline (speedup 1.0000x reference)
"""VQ codebook (nn_Codebook) Trainium2 kernel — 8-core data-parallel over tokens.

kernel(z, codebook) takes the FULL inputs (z [16,256,32,32] f32,
codebook [8192,256] f32) and returns the FULL output tuple
(z_q_out [16,256,32,32] f32, idx [16384] i32, loss scalar f32), matching the
jax reference.

Sharding: z is split along the batch dim (2 batches = 2048 tokens per core);
the codebook is replicated to all 8 cores (data-parallel, per sharding hint).

Device algorithm per core (all 16 m-tiles of 128 tokens):
- Scores S = z . e_norm^T computed on the tensor engine at full fp32 accuracy
  via an exact double-double style split: z = z_hi + z_lo with z_hi =
  round-to-12-bit-mantissa (fp32r) and z_lo the exact fp32 remainder (which is
  itself exactly representable in fp32r); same for the normalized codebook.
  S = z_hi.e_hi (fp32r matmul, full PE rate) + bf16(z_hi).bf16(e_lo) (bf16
  matmul) + z_lo.e_hi (fp32r matmul).  The only dropped/approximated terms are
  O(2^-22) relative — far below the smallest top-2 score gap — while running
  ~3x faster than the native fp32 matmul path (which needs 4 cycles/row).
  Argmax of S over the codebook equals the reference argmin of distances
  (per-token positive scaling of z does not change the argmax, so raw z rows
  feed the matmul; the codebook IS l2-normalized first).
- PSUM accumulates in [128,2048] groups, evacuated to SBUF by the scalar
  engine; the DVE max / max_index instructions produce the exact per-token
  argmax over each 4096-wide half, combined with a predicated select
  (ties prefer the lower index, matching jnp.argmin semantics).
- z_q = l2_norm(codebook)[idx] via indirect-DMA row gather of the normalized
  codebook; loss partials sum((z_q - z/||z||)^2) accumulate on the scalar
  engine.  Host does layout prep and the final scalar reduction across cores.
"""
import sys

try:
    import concourse.bass as bass
except ImportError:
    sys.path.insert(0, '/opt/trn_rl_repo')
    import concourse.bass as bass
import numpy as np
import ml_dtypes
import jax
from jax.sharding import Mesh, NamedSharding, PartitionSpec
from jax.experimental.shard_map import shard_map
import concourse.mybir as mybir
import concourse.tile as tile
from concourse import bacc, bass2jax
from concourse.bass import IndirectOffsetOnAxis
from concourse.bass2jax import install_neuronx_cc_hook, _bass_exec_p

F32 = mybir.dt.float32
F32R = mybir.dt.float32r
BF16 = mybir.dt.bfloat16
N_CORES = 8
TOK_PER_CORE = 2048
M_TILES = 16
K = 8192
D = 256


def fp32r_round(x):
    """Round fp32 to fp32r (12-bit mantissa), round-to-nearest-even."""
    b = x.view(np.uint32).astype(np.uint64)
    low = b & 0xFFF
    b2 = b & ~np.uint64(0xFFF)
    up = (low > 0x800) | ((low == 0x800) & (((b2 >> 12) & 1) == 1))
    return (b2 + np.where(up, np.uint64(0x1000), 0)).astype(np.uint32).view(np.float32)


def build_kernel(repeat=1):
    nc = bacc.Bacc("TRN2")
    zhr_d = nc.dram_tensor("zhr", [128, 2, TOK_PER_CORE], F32R, kind="ExternalInput")
    zlr_d = nc.dram_tensor("zlr", [128, 2, TOK_PER_CORE], F32R, kind="ExternalInput")
    zh16_d = nc.dram_tensor("zh16", [128, 2, TOK_PER_CORE], BF16, kind="ExternalInput")
    ehr_d = nc.dram_tensor("ehr", [128, 2, K], F32R, kind="ExternalInput")
    el16_d = nc.dram_tensor("el16", [128, 2, K], BF16, kind="ExternalInput")
    ztok_d = nc.dram_tensor("ztok", [128, M_TILES, D], F32, kind="ExternalInput")
    cbn_d = nc.dram_tensor("cbn", [K, D], F32, kind="ExternalInput")
    zq_d = nc.dram_tensor("zq", [128, M_TILES, D], F32, kind="ExternalOutput")
    idx_d = nc.dram_tensor("idx", [128, M_TILES], mybir.dt.int32, kind="ExternalOutput")
    ls_d = nc.dram_tensor("ls", [128, M_TILES], F32, kind="ExternalOutput")

    with tile.TileContext(nc) as tc:
        with tc.tile_pool(name="big", bufs=1) as big, \
             tc.tile_pool(name="sco", bufs=2) as sco, \
             tc.tile_pool(name="work", bufs=3) as work, \
             tc.tile_pool(name="zts", bufs=3) as zts, \
             tc.tile_pool(name="psum", bufs=2, space="PSUM") as pp:
            zhr = big.tile([128, 2, TOK_PER_CORE], F32R)
            zlr = big.tile([128, 2, TOK_PER_CORE], F32R)
            zh16 = big.tile([128, 2, TOK_PER_CORE], BF16)
            ehr = big.tile([128, 2, K], F32R)
            el16 = big.tile([128, 2, K], BF16)
            idx_st = big.tile([128, M_TILES], mybir.dt.int32)
            ls_st = big.tile([128, M_TILES], F32)
            nc.sync.dma_start(zhr[:], zhr_d[:])
            nc.sync.dma_start(zlr[:], zlr_d[:])
            nc.sync.dma_start(zh16[:], zh16_d[:])
            nc.sync.dma_start(ehr[:], ehr_d[:])
            nc.sync.dma_start(el16[:], el16_d[:])

            def main_body(_i):
                for m in range(M_TILES):
                    ms = bass.ts(m, 128)
                    t8s, i8s = [], []
                    for h in range(2):
                        sch = sco.tile([128, K // 2], F32, tag="sch", name=f"sch_{m}_{h}")
                        for g in range(2):
                            ps = pp.tile([128, 2048], F32, tag="ps", name=f"ps_{m}_{h}_{g}")
                            passes = [(zhr, ehr, 0), (zhr, ehr, 1), (zh16, el16, 0),
                                      (zh16, el16, 1), (zlr, ehr, 0), (zlr, ehr, 1)]
                            for kk in range(4):
                                k = h * 8 + g * 4 + kk
                                for pc, (lt, rt, cch) in enumerate(passes):
                                    nc.tensor.matmul(ps[:, bass.ts(kk, 512)],
                                                     lt[:, cch, ms],
                                                     rt[:, cch, bass.ts(k, 512)],
                                                     start=(pc == 0), stop=(pc == 5))
                            nc.scalar.copy(sch[:, bass.ts(g, 2048)], ps[:])
                        t8 = work.tile([128, 8], F32, tag=f"t8{h}", name=f"t8_{m}_{h}")
                        i8 = work.tile([128, 8], mybir.dt.uint32, tag=f"i8{h}",
                                       name=f"i8_{m}_{h}")
                        nc.vector.max(out=t8[:], in_=sch[:])
                        nc.vector.max_index(out=i8[:], in_max=t8[:], in_values=sch[:])
                        t8s.append(t8); i8s.append(i8)

                    # combine halves: idx = iA if vA >= vB else iB + 4096
                    idxc = work.tile([128, 1], mybir.dt.uint32, tag="idxc", name=f"idxc_{m}")
                    nc.vector.tensor_copy(idxc[:], i8s[0][:, 0:1])
                    ibo = work.tile([128, 1], mybir.dt.uint32, tag="ibo", name=f"ibo_{m}")
                    nc.vector.tensor_scalar(out=ibo[:], in0=i8s[1][:, 0:1], scalar1=K // 2,
                                            scalar2=None, op0=mybir.AluOpType.add)
                    bwin = work.tile([128, 1], mybir.dt.uint32, tag="bwin", name=f"bwin_{m}")
                    nc.vector.tensor_tensor(out=bwin[:], in0=t8s[1][:, 0:1],
                                            in1=t8s[0][:, 0:1], op=mybir.AluOpType.is_gt)
                    nc.vector.copy_predicated(idxc[:], bwin[:], ibo[:])
                    nc.vector.tensor_copy(idx_st[:, m:m + 1], idxc[:])

                    zq_n = work.tile([128, D], F32, tag="zq_n", name=f"zq_n_{m}")
                    nc.gpsimd.indirect_dma_start(
                        out=zq_n[:], out_offset=None, in_=cbn_d[:],
                        in_offset=IndirectOffsetOnAxis(ap=idx_st[:, m:m + 1], axis=0))

                    ztk = zts.tile([128, D], F32, tag="ztk", name=f"ztk_{m}")
                    nc.sync.dma_start(ztk[:], ztok_d[:, m, :])
                    sqs = work.tile([128, D], F32, tag="sqs", name=f"sqs_{m}")
                    ssq = work.tile([128, 1], F32, tag="ssq", name=f"ssq_{m}")
                    nc.scalar.activation(sqs[:], ztk[:],
                                         mybir.ActivationFunctionType.Square,
                                         accum_out=ssq[:])
                    nz = work.tile([128, 1], F32, tag="nz", name=f"nz_{m}")
                    nc.scalar.sqrt(nz[:], ssq[:])
                    rz = work.tile([128, 1], F32, tag="rz", name=f"rz_{m}")
                    nc.vector.reciprocal(rz[:], nz[:])
                    diff = work.tile([128, D], F32, tag="diff", name=f"diff_{m}")
                    # diff = z_tok/||z|| - zq_n up to sign (squared next)
                    nc.vector.scalar_tensor_tensor(
                        out=diff[:], in0=ztk[:], scalar=rz[:, 0:1], in1=zq_n[:],
                        op0=mybir.AluOpType.mult, op1=mybir.AluOpType.subtract)
                    sqd = work.tile([128, D], F32, tag="sqd", name=f"sqd_{m}")
                    nc.scalar.activation(sqd[:], diff[:],
                                         mybir.ActivationFunctionType.Square,
                                         accum_out=ls_st[:, m:m + 1])
                    nc.sync.dma_start(zq_d[:, m, :], zq_n[:])

            if repeat == 1:
                main_body(0)
            else:
                with tc.For_i(0, repeat, 1) as _it:
                    main_body(_it)

            nc.sync.dma_start(idx_d[:], idx_st[:])
            nc.sync.dma_start(ls_d[:], ls_st[:])
    nc.compile()
    return nc


def build_runner(nc, n_cores):
    """Compile the Bass module into a reusable jitted PJRT callable."""
    install_neuronx_cc_hook()
    partition_name = nc.partition_id_tensor.name if nc.partition_id_tensor else None
    in_names, out_names, out_avals, zero_outs = [], [], [], []
    for alloc in nc.m.functions[0].allocations:
        if not isinstance(alloc, mybir.MemoryLocationSet):
            continue
        name = alloc.memorylocations[0].name
        if alloc.kind == "ExternalInput":
            if name != partition_name:
                in_names.append(name)
        elif alloc.kind == "ExternalOutput":
            shape = tuple(alloc.tensor_shape)
            dtype = mybir.dt.np(alloc.dtype)
            out_names.append(name)
            out_avals.append(jax.core.ShapedArray(shape, dtype))
            zero_outs.append(np.zeros(shape, dtype))
    n_params = len(in_names)
    n_outs = len(out_avals)
    all_in_names = in_names + out_names + ([partition_name] if partition_name else [])
    REPLICATED = {"ehr", "el16", "cbn"}  # identical on every core: ship once

    def _body(*args):
        operands = list(args)
        if partition_name is not None:
            operands.append(bass2jax.partition_id_tensor())
        out = _bass_exec_p.bind(
            *operands,
            out_avals=tuple(out_avals),
            in_names=tuple(all_in_names),
            out_names=tuple(out_names),
            lowering_input_output_aliases=(),
            sim_require_finite=True,
            sim_require_nnan=True,
            nc=nc,
        )
        return tuple(out)

    devices = jax.devices()[:n_cores]
    mesh = Mesh(np.asarray(devices), ("core",))
    in_specs = tuple(
        PartitionSpec() if name in REPLICATED else PartitionSpec("core")
        for name in in_names) + (PartitionSpec("core"),) * n_outs
    out_specs = (PartitionSpec("core"),) * n_outs
    sharded = jax.jit(
        shard_map(_body, mesh=mesh, in_specs=in_specs, out_specs=out_specs,
                  check_rep=False),
        keep_unused=True,
    )

    def stage(in_maps):
        args = []
        for i, name in enumerate(in_names):
            if name in REPLICATED:
                args.append(np.asarray(in_maps[0][name]))
            else:
                args.append(np.concatenate(
                    [np.asarray(in_maps[c][name]) for c in range(n_cores)], axis=0))
        args += [np.zeros((n_cores * z.shape[0], *z.shape[1:]), z.dtype)
                 for z in zero_outs]
        return [jax.device_put(a, NamedSharding(mesh, spec))
                for a, spec in zip(args, in_specs)]

    def exec_staged(staged):
        out = sharded(*staged)
        jax.block_until_ready(out)
        return [
            {name: np.asarray(out[i]).reshape(n_cores, *out_avals[i].shape)[c]
             for i, name in enumerate(out_names)}
            for c in range(n_cores)
        ]

    def run(in_maps):
        return exec_staged(stage(in_maps))

    run.stage = stage
    run.exec_staged = exec_staged
    return run


_CACHE = {}


def _get_runner():
    if "run" not in _CACHE:
        nc = build_kernel()
        _CACHE["run"] = build_runner(nc, N_CORES)
    return _CACHE["run"]


def prep_inputs(z, codebook):
    """Host-side sharding + layout prep (transposes, l2-norm, hi/lo split)."""
    z = np.asarray(z, dtype=np.float32)
    cb = np.asarray(codebook, dtype=np.float32)
    n = np.sqrt((cb * cb).sum(axis=1, keepdims=True, dtype=np.float32))
    e_norm = cb / np.maximum(n, np.float32(1e-12))

    eT = np.ascontiguousarray(e_norm.T)                 # [256, 8192]
    e_hi = fp32r_round(eT)
    e_lo16 = (eT - e_hi).astype(ml_dtypes.bfloat16)

    def chunk(x):
        return np.ascontiguousarray(x.reshape(2, 128, x.shape[-1]).transpose(1, 0, 2))

    ehr = chunk(e_hi)
    el16 = chunk(e_lo16)
    in_maps = []
    for c in range(N_CORES):
        zc = z[2 * c:2 * c + 2]                          # [2,256,32,32]
        zT = np.ascontiguousarray(np.moveaxis(zc, 1, 0).reshape(256, TOK_PER_CORE))
        z_hi = fp32r_round(zT)
        z_lo = (zT - z_hi).astype(np.float32)
        z_h16 = z_hi.astype(ml_dtypes.bfloat16)
        ztokc = np.ascontiguousarray(
            zc.transpose(0, 2, 3, 1).reshape(TOK_PER_CORE, D)
            .reshape(M_TILES, 128, D).transpose(1, 0, 2))
        in_maps.append({"zhr": chunk(z_hi), "zlr": chunk(z_lo), "zh16": chunk(z_h16),
                        "ehr": ehr, "el16": el16, "ztok": ztokc, "cbn": e_norm})
    return in_maps


def assemble(results):
    zq_flat = np.empty((16384, D), np.float32)
    idx_flat = np.empty((16384,), np.int32)
    total = 0.0
    for c, r in enumerate(results):
        zq_flat[c * TOK_PER_CORE:(c + 1) * TOK_PER_CORE] = \
            r["zq"].transpose(1, 0, 2).reshape(TOK_PER_CORE, D)
        idx_flat[c * TOK_PER_CORE:(c + 1) * TOK_PER_CORE] = \
            r["idx"].transpose(1, 0).reshape(TOK_PER_CORE)
        total += r["ls"].astype(np.float64).sum()
    z_q_out = np.ascontiguousarray(
        zq_flat.reshape(16, 32, 32, D).transpose(0, 3, 1, 2))
    loss = np.float32(1.25 * total / (16384 * D))
    return z_q_out, idx_flat, loss


def kernel(z, codebook):
    in_maps = prep_inputs(z, codebook)
    run = _get_runner()
    results = run(in_maps)
    return assemble(results)


# revision 5
# speedup vs baseline: 7.7010x; 7.7010x over previous
"""VQ codebook (nn_Codebook) Trainium2 kernel — 8-core data-parallel over tokens.

kernel(z, codebook) takes the FULL inputs (z [16,256,32,32] f32,
codebook [8192,256] f32) and returns the FULL output tuple
(z_q_out [16,256,32,32] f32, idx [16384] i32, loss scalar f32), matching the
jax reference.

Sharding: z is split along the batch dim (2 batches = 2048 tokens per core);
the codebook is replicated to all 8 cores (data-parallel, per sharding hint).

Device algorithm per core (all 16 m-tiles of 128 tokens):
- Scores S = z . e_norm^T computed on the tensor engine at full fp32 accuracy
  via an exact double-double style split: z = z_hi + z_lo with z_hi =
  round-to-12-bit-mantissa (fp32r) and z_lo the exact fp32 remainder (which is
  itself exactly representable in fp32r); same for the normalized codebook.
  S = z_hi.e_hi (fp32r matmul, full PE rate) + bf16(z_hi).bf16(e_lo) (bf16
  matmul) + z_lo.e_hi (fp32r matmul).  The only dropped/approximated terms are
  O(2^-22) relative — far below the smallest top-2 score gap — while running
  ~3x faster than the native fp32 matmul path (which needs 4 cycles/row).
  Argmax of S over the codebook equals the reference argmin of distances
  (per-token positive scaling of z does not change the argmax, so raw z rows
  feed the matmul; the codebook IS l2-normalized first).
- PSUM accumulates in [128,2048] groups, evacuated to SBUF by the scalar
  engine; the DVE max / max_index instructions produce the exact per-token
  argmax over each 4096-wide half, combined with a predicated select
  (ties prefer the lower index, matching jnp.argmin semantics).
- z_q = l2_norm(codebook)[idx] via indirect-DMA row gather of the normalized
  codebook; loss partials sum((z_q - z/||z||)^2) accumulate on the scalar
  engine.  Host does layout prep and the final scalar reduction across cores.
"""
import sys

try:
    import concourse.bass as bass
except ImportError:
    sys.path.insert(0, '/opt/trn_rl_repo')
    import concourse.bass as bass
import numpy as np
import ml_dtypes
import jax
from jax.sharding import Mesh, NamedSharding, PartitionSpec
from jax.experimental.shard_map import shard_map
import concourse.mybir as mybir
import concourse.tile as tile
from concourse import bacc, bass2jax
from concourse.bass import IndirectOffsetOnAxis
from concourse.bass2jax import install_neuronx_cc_hook, _bass_exec_p

F32 = mybir.dt.float32
F32R = mybir.dt.float32r
BF16 = mybir.dt.bfloat16
N_CORES = 8
TOK_PER_CORE = 2048
M_TILES = 16
K = 8192
D = 256


def fp32r_round(x):
    """Round fp32 to fp32r (12-bit mantissa), round-to-nearest-even."""
    b = x.view(np.uint32).astype(np.uint64)
    low = b & 0xFFF
    b2 = b & ~np.uint64(0xFFF)
    up = (low > 0x800) | ((low == 0x800) & (((b2 >> 12) & 1) == 1))
    return (b2 + np.where(up, np.uint64(0x1000), 0)).astype(np.uint32).view(np.float32)


def build_kernel(repeat=1):
    nc = bacc.Bacc("TRN2")
    zhr_d = nc.dram_tensor("zhr", [128, 2, TOK_PER_CORE], F32R, kind="ExternalInput")
    zlr_d = nc.dram_tensor("zlr", [128, 2, TOK_PER_CORE], F32R, kind="ExternalInput")
    zh16_d = nc.dram_tensor("zh16", [128, 2, TOK_PER_CORE], BF16, kind="ExternalInput")
    ehr_d = nc.dram_tensor("ehr", [128, 2, K], F32R, kind="ExternalInput")
    el16_d = nc.dram_tensor("el16", [128, 2, K], BF16, kind="ExternalInput")
    ztok_d = nc.dram_tensor("ztok", [128, M_TILES, D], F32, kind="ExternalInput")
    cbn_d = nc.dram_tensor("cbn", [K, D], F32, kind="ExternalInput")
    zq_d = nc.dram_tensor("zq", [128, M_TILES, D], F32, kind="ExternalOutput")
    idx_d = nc.dram_tensor("idx", [128, M_TILES], mybir.dt.int32, kind="ExternalOutput")
    ls_d = nc.dram_tensor("ls", [128, M_TILES], F32, kind="ExternalOutput")

    with tile.TileContext(nc) as tc:
        with tc.tile_pool(name="big", bufs=1) as big, \
             tc.tile_pool(name="sco", bufs=2) as sco, \
             tc.tile_pool(name="work", bufs=3) as work, \
             tc.tile_pool(name="zts", bufs=3) as zts, \
             tc.tile_pool(name="psum", bufs=2, space="PSUM") as pp:
            zhr = big.tile([128, 2, TOK_PER_CORE], F32R)
            zlr = big.tile([128, 2, TOK_PER_CORE], F32R)
            zh16 = big.tile([128, 2, TOK_PER_CORE], BF16)
            ehr = big.tile([128, 2, K], F32R)
            el16 = big.tile([128, 2, K], BF16)
            idx_st = big.tile([128, M_TILES], mybir.dt.int32)
            ls_st = big.tile([128, M_TILES], F32)
            nc.sync.dma_start(zhr[:], zhr_d[:])
            nc.sync.dma_start(zlr[:], zlr_d[:])
            nc.sync.dma_start(zh16[:], zh16_d[:])
            nc.sync.dma_start(ehr[:], ehr_d[:])
            nc.sync.dma_start(el16[:], el16_d[:])

            def main_body(_i):
                for m in range(M_TILES):
                    ms = bass.ts(m, 128)
                    t8s, i8s = [], []
                    for h in range(2):
                        sch = sco.tile([128, K // 2], F32, tag="sch", name=f"sch_{m}_{h}")
                        for g in range(2):
                            ps = pp.tile([128, 2048], F32, tag="ps", name=f"ps_{m}_{h}_{g}")
                            passes = [(zhr, ehr, 0), (zhr, ehr, 1), (zh16, el16, 0),
                                      (zh16, el16, 1), (zlr, ehr, 0), (zlr, ehr, 1)]
                            for kk in range(4):
                                k = h * 8 + g * 4 + kk
                                for pc, (lt, rt, cch) in enumerate(passes):
                                    nc.tensor.matmul(ps[:, bass.ts(kk, 512)],
                                                     lt[:, cch, ms],
                                                     rt[:, cch, bass.ts(k, 512)],
                                                     start=(pc == 0), stop=(pc == 5))
                            nc.scalar.copy(sch[:, bass.ts(g, 2048)], ps[:])
                        t8 = work.tile([128, 8], F32, tag=f"t8{h}", name=f"t8_{m}_{h}")
                        i8 = work.tile([128, 8], mybir.dt.uint32, tag=f"i8{h}",
                                       name=f"i8_{m}_{h}")
                        nc.vector.max(out=t8[:], in_=sch[:])
                        nc.vector.max_index(out=i8[:], in_max=t8[:], in_values=sch[:])
                        t8s.append(t8); i8s.append(i8)

                    # combine halves: idx = iA if vA >= vB else iB + 4096
                    idxc = work.tile([128, 1], mybir.dt.uint32, tag="idxc", name=f"idxc_{m}")
                    nc.vector.tensor_copy(idxc[:], i8s[0][:, 0:1])
                    ibo = work.tile([128, 1], mybir.dt.uint32, tag="ibo", name=f"ibo_{m}")
                    nc.vector.tensor_scalar(out=ibo[:], in0=i8s[1][:, 0:1], scalar1=K // 2,
                                            scalar2=None, op0=mybir.AluOpType.add)
                    bwin = work.tile([128, 1], mybir.dt.uint32, tag="bwin", name=f"bwin_{m}")
                    nc.vector.tensor_tensor(out=bwin[:], in0=t8s[1][:, 0:1],
                                            in1=t8s[0][:, 0:1], op=mybir.AluOpType.is_gt)
                    nc.vector.copy_predicated(idxc[:], bwin[:], ibo[:])
                    nc.vector.tensor_copy(idx_st[:, m:m + 1], idxc[:])

                    zq_n = work.tile([128, D], F32, tag="zq_n", name=f"zq_n_{m}")
                    nc.gpsimd.indirect_dma_start(
                        out=zq_n[:], out_offset=None, in_=cbn_d[:],
                        in_offset=IndirectOffsetOnAxis(ap=idx_st[:, m:m + 1], axis=0))

                    ztk = zts.tile([128, D], F32, tag="ztk", name=f"ztk_{m}")
                    nc.sync.dma_start(ztk[:], ztok_d[:, m, :])
                    sqs = work.tile([128, D], F32, tag="sqs", name=f"sqs_{m}")
                    ssq = work.tile([128, 1], F32, tag="ssq", name=f"ssq_{m}")
                    nc.scalar.activation(sqs[:], ztk[:],
                                         mybir.ActivationFunctionType.Square,
                                         accum_out=ssq[:])
                    nz = work.tile([128, 1], F32, tag="nz", name=f"nz_{m}")
                    nc.scalar.sqrt(nz[:], ssq[:])
                    rz = work.tile([128, 1], F32, tag="rz", name=f"rz_{m}")
                    nc.vector.reciprocal(rz[:], nz[:])
                    diff = work.tile([128, D], F32, tag="diff", name=f"diff_{m}")
                    # diff = z_tok/||z|| - zq_n up to sign (squared next)
                    nc.vector.scalar_tensor_tensor(
                        out=diff[:], in0=ztk[:], scalar=rz[:, 0:1], in1=zq_n[:],
                        op0=mybir.AluOpType.mult, op1=mybir.AluOpType.subtract)
                    sqd = work.tile([128, D], F32, tag="sqd", name=f"sqd_{m}")
                    nc.scalar.activation(sqd[:], diff[:],
                                         mybir.ActivationFunctionType.Square,
                                         accum_out=ls_st[:, m:m + 1])
                    nc.sync.dma_start(zq_d[:, m, :], zq_n[:])

            if repeat == 1:
                main_body(0)
            else:
                with tc.For_i(0, repeat, 1) as _it:
                    main_body(_it)

            nc.sync.dma_start(idx_d[:], idx_st[:])
            nc.sync.dma_start(ls_d[:], ls_st[:])
    nc.compile()
    return nc


def build_runner(nc, n_cores):
    """Compile the Bass module into a reusable jitted PJRT callable."""
    install_neuronx_cc_hook()
    partition_name = nc.partition_id_tensor.name if nc.partition_id_tensor else None
    in_names, out_names, out_avals, zero_outs = [], [], [], []
    for alloc in nc.m.functions[0].allocations:
        if not isinstance(alloc, mybir.MemoryLocationSet):
            continue
        name = alloc.memorylocations[0].name
        if alloc.kind == "ExternalInput":
            if name != partition_name:
                in_names.append(name)
        elif alloc.kind == "ExternalOutput":
            shape = tuple(alloc.tensor_shape)
            dtype = mybir.dt.np(alloc.dtype)
            out_names.append(name)
            out_avals.append(jax.core.ShapedArray(shape, dtype))
            zero_outs.append(np.zeros(shape, dtype))
    n_params = len(in_names)
    n_outs = len(out_avals)
    all_in_names = in_names + out_names + ([partition_name] if partition_name else [])
    REPLICATED = set()  # replicated specs proved slower via axon; ship per-core copies

    def _body(*args):
        operands = list(args)
        if partition_name is not None:
            operands.append(bass2jax.partition_id_tensor())
        out = _bass_exec_p.bind(
            *operands,
            out_avals=tuple(out_avals),
            in_names=tuple(all_in_names),
            out_names=tuple(out_names),
            lowering_input_output_aliases=(),
            sim_require_finite=True,
            sim_require_nnan=True,
            nc=nc,
        )
        return tuple(out)

    devices = jax.devices()[:n_cores]
    mesh = Mesh(np.asarray(devices), ("core",))
    in_specs = tuple(
        PartitionSpec() if name in REPLICATED else PartitionSpec("core")
        for name in in_names) + (PartitionSpec("core"),) * n_outs
    out_specs = (PartitionSpec("core"),) * n_outs
    sharded = jax.jit(
        shard_map(_body, mesh=mesh, in_specs=in_specs, out_specs=out_specs,
                  check_rep=False),
        keep_unused=True,
    )

    def stage(in_maps):
        args = []
        for i, name in enumerate(in_names):
            if name in REPLICATED:
                args.append(np.asarray(in_maps[0][name]))
            else:
                args.append(np.concatenate(
                    [np.asarray(in_maps[c][name]) for c in range(n_cores)], axis=0))
        args += [np.zeros((n_cores * z.shape[0], *z.shape[1:]), z.dtype)
                 for z in zero_outs]
        return [jax.device_put(a, NamedSharding(mesh, spec))
                for a, spec in zip(args, in_specs)]

    def exec_staged(staged):
        out = sharded(*staged)
        jax.block_until_ready(out)
        return [
            {name: np.asarray(out[i]).reshape(n_cores, *out_avals[i].shape)[c]
             for i, name in enumerate(out_names)}
            for c in range(n_cores)
        ]

    def run(in_maps):
        return exec_staged(stage(in_maps))

    run.stage = stage
    run.exec_staged = exec_staged
    return run


_CACHE = {}


def _get_runner():
    if "run" not in _CACHE:
        nc = build_kernel()
        _CACHE["run"] = build_runner(nc, N_CORES)
    return _CACHE["run"]


def prep_inputs(z, codebook):
    """Host-side sharding + layout prep (transposes, l2-norm, hi/lo split)."""
    z = np.asarray(z, dtype=np.float32)
    cb = np.asarray(codebook, dtype=np.float32)
    n = np.sqrt((cb * cb).sum(axis=1, keepdims=True, dtype=np.float32))
    e_norm = cb / np.maximum(n, np.float32(1e-12))

    eT = np.ascontiguousarray(e_norm.T)                 # [256, 8192]
    e_hi = fp32r_round(eT)
    e_lo16 = (eT - e_hi).astype(ml_dtypes.bfloat16)

    def chunk(x):
        return np.ascontiguousarray(x.reshape(2, 128, x.shape[-1]).transpose(1, 0, 2))

    ehr = chunk(e_hi)
    el16 = chunk(e_lo16)
    in_maps = []
    for c in range(N_CORES):
        zc = z[2 * c:2 * c + 2]                          # [2,256,32,32]
        zT = np.ascontiguousarray(np.moveaxis(zc, 1, 0).reshape(256, TOK_PER_CORE))
        z_hi = fp32r_round(zT)
        z_lo = (zT - z_hi).astype(np.float32)
        z_h16 = z_hi.astype(ml_dtypes.bfloat16)
        ztokc = np.ascontiguousarray(
            zc.transpose(0, 2, 3, 1).reshape(TOK_PER_CORE, D)
            .reshape(M_TILES, 128, D).transpose(1, 0, 2))
        in_maps.append({"zhr": chunk(z_hi), "zlr": chunk(z_lo), "zh16": chunk(z_h16),
                        "ehr": ehr, "el16": el16, "ztok": ztokc, "cbn": e_norm})
    return in_maps


def assemble(results):
    zq_flat = np.empty((16384, D), np.float32)
    idx_flat = np.empty((16384,), np.int32)
    total = 0.0
    for c, r in enumerate(results):
        zq_flat[c * TOK_PER_CORE:(c + 1) * TOK_PER_CORE] = \
            r["zq"].transpose(1, 0, 2).reshape(TOK_PER_CORE, D)
        idx_flat[c * TOK_PER_CORE:(c + 1) * TOK_PER_CORE] = \
            r["idx"].transpose(1, 0).reshape(TOK_PER_CORE)
        total += r["ls"].astype(np.float64).sum()
    z_q_out = np.ascontiguousarray(
        zq_flat.reshape(16, 32, 32, D).transpose(0, 3, 1, 2))
    loss = np.float32(1.25 * total / (16384 * D))
    return z_q_out, idx_flat, loss


def kernel(z, codebook):
    in_maps = prep_inputs(z, codebook)
    run = _get_runner()
    results = run(in_maps)
    return assemble(results)
